# revision 37
# baseline (speedup 1.0000x reference)
"""Trainium2 Bass kernel for nn_BasicTransformerBlock (self-contained).

Sharding: sequence-parallel. 8 cores = 2 batch groups x 4 cores; each core
owns 512 tokens of one batch element. K^T and V are AllGathered (fp8,
x16-scaled) within each 4-core group so every core attends over the full
2048-token context of its batch element.

Key design points per core:
  - LayerNorm gains/scales are folded into the weights host-side; xln is
    produced transposed (PE transpose) in fp8 kd-pair layout [128, 2, 512].
  - All Q/K/V/O projections run as fp8 DoubleRow matmuls (2 contraction
    tiles per instruction); scale factors are folded into the weights and
    undone on the psum->SBUF copies.
  - Scores: per head-pair, s=0/s=1 run as row-tiled concurrent matmul pairs
    (K=64 each, rows 0-63 / 64-127 of the PE array).
  - Softmax exp is split across engines: s=0 on ACT (native Exp -> fp8),
    s=1 on DVE via a Schraudolph bit-trick (int8 output = fp8e4m3 bits of
    exp, tensor_scalar mult+add), with every 4th round's s=1 moved to ACT
    to balance engine load.
  - AV + softmax denominator are fused: V tiles hold per-head
    [V(64) | ones(64)] blocks in DoubleRow j-pair layout, so one fp8 DR
    matmul per (head, j-pair) yields AV in psum rows 0:64 and the
    denominator (x64 copies) in rows 64:128.
  - Epilogue per head: DVE copy (shifted) -> reciprocal_approx_fast ->
    multiply into fp8 attnT pairs; out-projection is fp8 DoubleRow with the
    1/256 net scale folded into the residual add (scalar_tensor_tensor).
  - The GEGLU FF stays bf16 (fp8 there costs ~2e-2 max-rel error, over the
    tolerance); FF1/FF2 are dense bf16 matmul streams.
"""


from contextlib import ExitStack

import numpy as np

import concourse.bass as bass
import concourse.mybir as mybir
import concourse.tile as tile
from concourse.tile_rust import add_dep_helper
from concourse import bacc
from concourse.masks import make_identity

F32 = mybir.dt.float32
BF16 = mybir.dt.bfloat16
I8 = mybir.dt.int8
FP8 = mybir.dt.float8e4
AX = mybir.AxisListType.X
AF = mybir.ActivationFunctionType
ALU = mybir.AluOpType

# Schraudolph fast-exp: int8 bits of fp8e4m3(exp(s)) ~= A_EXP8*s + B_EXP8
A_EXP8 = 8.0 / float(np.log(2.0))
B_EXP8 = 8.0 * (7.0 + 0.215)

D = 1024          # model dim
HEADS = 16
DH = 64
FF = 4096         # ff inner (per half)
EPS = 1e-5
P = 128


class Ctx:
    """Holds the bass handles shared across emit stages."""


def build(group: int, tok: int, use_bias: bool = False):
    """group: cores per batch group (1 = no collective, 4 = real).
    tok: local tokens per core (512)."""
    ntok = group * tok
    TT = tok // P          # local token tiles (4)
    JT = ntok // P         # context token tiles (16 when group=4)

    nc = bacc.Bacc("TRN2", target_bir_lowering=False, debug=False,
                   num_devices=8)

    c = Ctx()
    c.nc = nc
    c.group = group
    c.tok = tok
    c.ntok = ntok
    c.TT = TT
    c.JT = JT
    c.use_bias = use_bias

    # ---- I/O ----
    c.x_in = nc.dram_tensor("x", [tok, D], F32, kind="ExternalInput")
    c.y_out = nc.dram_tensor("y", [tok, D], F32, kind="ExternalOutput")
    w = {}
    for i in (1, 2):
        for nm in ("wq", "wk", "wv", "wo"):
            w[f"{nm}{i}"] = nc.dram_tensor(f"{nm}{i}", [4, P, 2, D], FP8,
                                           kind="ExternalInput")
    w["wf1"] = nc.dram_tensor("wf1", [32, P, 2048], BF16, kind="ExternalInput")
    w["wf2"] = nc.dram_tensor("wf2", [FF, D], BF16, kind="ExternalInput")
    if use_bias:
        # per-projection bias vectors (LN beta pushed through W, plus the
        # projection's own bias where it exists)
        for i in (1, 2):
            for nmv in ("cq", "ck"):
                w[f"{nmv}{i}"] = nc.dram_tensor(f"{nmv}{i}", [P, 8], F32,
                                                kind="ExternalInput")
            for nmv in ("cv", "bo"):
                w[f"{nmv}{i}"] = nc.dram_tensor(f"{nmv}{i}", [1, D], BF16,
                                                kind="ExternalInput")
        w["c1"] = nc.dram_tensor("c1", [1, 2 * FF], BF16, kind="ExternalInput")
        w["bf2"] = nc.dram_tensor("bf2", [1, D], BF16, kind="ExternalInput")
    c.w = w

    with ExitStack() as stack:
        tc = stack.enter_context(tile.TileContext(nc))
        c.tc = tc

        const = stack.enter_context(tc.tile_pool(name="const", bufs=1))
        c.identity = const.tile([P, P], BF16, name="identity")
        make_identity(nc, c.identity)
        c.ones64 = const.tile([P, 64], BF16, name="ones64")
        nc.vector.memset(c.ones64, 1.0)
        c.eps_tile = const.tile([P, 1], F32, name="eps_tile")
        nc.vector.memset(c.eps_tile, EPS)
        if use_bias:
            c.ones_bf = const.tile([1, tok], BF16, name="ones_bf")
            nc.vector.memset(c.ones_bf, 1.0)
            c.bias_sb = {}
            for key, t in w.items():
                if key[:2] in ("cq", "ck"):
                    bt = const.tile([P, 8], F32, name=f"sb_{key}")
                elif key[:2] in ("cv", "bo", "bf"):
                    bt = const.tile([1, D], BF16, name=f"sb_{key}")
                else:
                    continue
                nc.sync.dma_start(bt, t[:, :])
                c.bias_sb[key] = bt
            bt = const.tile([1, 2 * FF], BF16, name="sb_c1")
            nc.sync.dma_start(bt, w["c1"][:, :])
            c.bias_sb["c1"] = bt

        xres_pool = stack.enter_context(tc.tile_pool(name="xres", bufs=1))
        c.xres = [xres_pool.tile([P, D], F32, name=f"xres{t}") for t in range(TT)]
        for t in range(TT):
            nc.sync.dma_start(c.xres[t], c.x_in[t * P:(t + 1) * P, :])

        # DRAM bounce buffers: half-sized K/V allgathers (per attention)
        if group > 1:
            ke = D * tok               # whole K^T
            vh = tok * (HEADS // 2) * DH   # half of V (no ones gathered)
            dram = stack.enter_context(
                tc.tile_pool(name="dram", bufs=1, space="DRAM"))
            c.k_in = [dram.tile([ke], FP8, name=f"k_in{i}")
                      for i in (0, 1)]
            c.k_out = [dram.tile([group * ke], FP8, name=f"k_out{i}")
                       for i in (0, 1)]
            c.v_in = [dram.tile([2 * vh], FP8, name=f"v_in{i}")
                      for i in (0, 1)]
            c.v_out = [dram.tile([group * 2 * vh], FP8, name=f"v_out{i}")
                       for i in (0, 1)]

        emit_attn(c, 1)
        emit_attn(c, 2)
        emit_ff(c)

        for t in range(TT):
            nc.sync.dma_start(c.y_out[t * P:(t + 1) * P, :], c.xres[t])

    nc.compile()
    return nc


def emit_ln_transpose(c, outer, name, dtype=BF16, scale=1.0, pairs=False):
    """LayerNorm xres (gain/bias pre-folded into weights) and produce
    xlnT: transposed normalized x, optionally scaled and in kd-pair
    layout ([128, 2, tok] x4) for DoubleRow consumers.
    xlnT tiles live in `outer`; scratch pools are closed on return."""
    nc, tc = c.nc, c.tc
    TT = c.TT

    xlnT_pool = outer.enter_context(tc.tile_pool(name=f"{name}_xlnT", bufs=1))
    if pairs:
        xlnT = [xlnT_pool.tile([P, 2, c.tok], dtype, name=f"{name}_xlnT{d}")
                for d in range(4)]
    else:
        xlnT = [xlnT_pool.tile([P, c.tok], dtype, name=f"{name}_xlnT{d}")
                for d in range(8)]

    with ExitStack() as ph:
        pool = ph.enter_context(tc.tile_pool(name=f"{name}_ln", bufs=2))
        psum_sq = ph.enter_context(
            tc.tile_pool(name=f"{name}_psq", bufs=2, space="PSUM"))
        psum_tr = ph.enter_context(
            tc.tile_pool(name=f"{name}_ptr", bufs=4, space="PSUM"))

        for t in range(TT):
            xr = c.xres[t]
            # var = E[x^2] - mu^2: sum and sumsq run concurrently (DVE / ACT)
            ssum = pool.tile([P, 1], F32, tag="ssum", name=f"{name}_ssum{t}")
            nc.vector.reduce_sum(ssum, xr, axis=AX)
            sq_sink = psum_sq.tile([P, D], F32, tag="sq", name=f"{name}_sq{t}")
            sumsq = pool.tile([P, 1], F32, tag="sumsq", name=f"{name}_vs{t}")
            nc.scalar.activation(sq_sink, xr, AF.Square, accum_out=sumsq)
            mu = pool.tile([P, 1], F32, tag="mu", name=f"{name}_mu{t}")
            nc.vector.tensor_scalar_mul(mu, ssum, 1.0 / D)
            musq = pool.tile([P, 1], F32, tag="musq", name=f"{name}_msq{t}")
            nc.vector.tensor_mul(musq, mu, mu)
            bvar = pool.tile([P, 1], F32, tag="bvar", name=f"{name}_bv{t}")
            nc.vector.tensor_scalar(bvar, musq, -1.0, EPS,
                                    op0=ALU.mult, op1=ALU.add)
            std = pool.tile([P, 1], F32, tag="std", name=f"{name}_std{t}")
            nc.scalar.activation(std, sumsq, AF.Sqrt, bias=bvar,
                                 scale=1.0 / D)
            rstd = pool.tile([P, 1], F32, tag="rstd", name=f"{name}_rstd{t}")
            nc.vector.reciprocal(rstd, std)
            if scale != 1.0:
                nc.vector.tensor_scalar_mul(rstd, rstd, scale)
            nmr = pool.tile([P, 1], F32, tag="nmr", name=f"{name}_nmr{t}")
            nc.vector.tensor_mul(nmr, mu, rstd)
            nc.vector.tensor_scalar_mul(nmr, nmr, -1.0)
            xln = pool.tile([P, D], BF16, tag="xln", name=f"{name}_xln{t}")
            nc.vector.tensor_scalar(xln, xr, rstd, nmr,
                                    op0=ALU.mult, op1=ALU.add)
            for dc in range(8):
                tp = psum_tr.tile([P, P], BF16, tag="tp",
                                  name=f"{name}_tp{t}_{dc}")
                nc.tensor.transpose(tp, xln[:, dc * P:(dc + 1) * P],
                                    c.identity)
                if pairs:
                    dst = xlnT[dc // 2][:, dc % 2, t * P:(t + 1) * P]
                else:
                    dst = xlnT[dc][:, t * P:(t + 1) * P]
                nc.vector.tensor_copy(dst, tp)
    return xlnT


def proj_fm(c, ph, name, xlnT4, w_dram, out_tiles, bias_key=None,
            out_scale=1.0, post_m=None):
    """Feature-major projection via fp8 DoubleRow:
    out^T[m] [128, tok] = out_scale * W-pair.T @ xlnT4 (+ bias)."""
    nc, tc = c.nc, c.tc
    DRm = mybir.MatmulPerfMode.DoubleRow
    with ExitStack() as sub:
        wpool = sub.enter_context(tc.tile_pool(name=f"{name}_w", bufs=1))
        w_tiles = []
        for kp in range(4):
            wt = wpool.tile([P, 2, D], FP8, name=f"{name}_w{kp}")
            nc.sync.dma_start(wt, w_dram[kp, :, :, :])
            w_tiles.append(wt)
        psum = sub.enter_context(
            tc.tile_pool(name=f"{name}_ps", bufs=3, space="PSUM"))
        has_bias = c.use_bias and bias_key is not None
        for m in range(8):
            ps = psum.tile([P, c.tok], F32, tag="proj", name=f"{name}_ps{m}")
            for kp in range(4):
                nc.tensor.matmul(ps,
                                 lhsT=w_tiles[kp][:, :, m * P:(m + 1) * P],
                                 rhs=xlnT4[kp], start=(kp == 0),
                                 stop=(kp == 3), perf_mode=DRm)
            if has_bias:
                nc.vector.tensor_scalar(
                    out_tiles[m], ps, out_scale,
                    c.bias_sb[bias_key][:, m:m + 1],
                    op0=ALU.mult, op1=ALU.add)
            else:
                nc.vector.tensor_scalar_mul(out_tiles[m], ps, out_scale)
            if post_m is not None:
                post_m(m)


def emit_attn(c, idx):
    nc, tc = c.nc, c.tc
    name = f"a{idx}"
    TT, JT, tok = c.TT, c.JT, c.tok
    RG = [[0, 1, 2, 3], [4, 5, 6, 7]]
    vw = HEADS * DH          # vstage: 16 plain head blocks
    vhw = (HEADS // 2) * DH  # gathered width per head-half (no ones)

    with ExitStack() as ph:
        xlnT4 = emit_ln_transpose(c, ph, name, dtype=FP8, scale=4.0,
                                  pairs=True)

        kfull_pool = ph.enter_context(tc.tile_pool(name=f"{name}_kf", bufs=1))
        kT_m = [kfull_pool.tile([P, c.ntok], FP8, name=f"{name}_kTm{m}")
                for m in range(8)]
        # V tiles: per head a [V(64) | ones(64)] 128-col block, j-tile
        # pairs interleaved on the ko dim for DoubleRow AV: one DR matmul
        # per (head, j-pair) computes AV (out rows 0:64) AND the softmax
        # denominator x64 (out rows 64:128).
        vones_hr = [[kfull_pool.tile([P, TT // 2, 2, 8 * P], FP8,
                                     name=f"{name}_vo{h}_{r}")
                     for r in range(c.group if c.group > 1 else 1)]
                    for h in (0, 1)]
        qT = [kfull_pool.tile([P, tok], BF16, name=f"{name}_qT{m}")
              for m in range(8)]
        for h in (0, 1):
            for r in range(c.group if c.group > 1 else 1):
                for tp in range(TT // 2):
                    for ko in range(2):
                        nc.gpsimd.memset(
                            vones_hr[h][r][:, tp, ko, :].rearrange(
                                "p (hh e) -> p hh e", e=P)[:, :, 64:128],
                            1.0)

        stage_stack = ExitStack()
        kv_pool = stage_stack.enter_context(
            tc.tile_pool(name=f"{name}_kv", bufs=1))
        kT_loc = [kv_pool.tile([P, tok], FP8, name=f"{name}_kTl{m}")
                  for m in range(8)]
        vstage = [kv_pool.tile([P, vw], FP8, name=f"{name}_vst{t}")
                  for t in range(TT)]

        # --- K^T projection; kick the K allgather when it completes ---
        proj_fm(c, ph, f"{name}_kproj", xlnT4, c.w[f"wk{idx}"], kT_loc,
                bias_key=f"ck{idx}", out_scale=0.25)
        if c.group > 1:
            k_in = c.k_in[idx - 1]
            for m in range(8):
                nc.sync.dma_start(
                    k_in[m * P * tok:(m + 1) * P * tok]
                    .rearrange("(p f) -> p f", f=tok), kT_loc[m])
            nc.gpsimd.collective_compute(
                "AllGather", ALU.bypass, replica_groups=RG,
                ins=[c.k_in[idx - 1][:]], outs=[c.k_out[idx - 1][:]])

        # --- V(+ones) projection, n-major so head-half 0 finishes first ---
        DRm = mybir.MatmulPerfMode.DoubleRow
        with ExitStack() as sub:
            wpool = sub.enter_context(tc.tile_pool(name=f"{name}_wvp", bufs=1))
            wv_tiles = []
            for kp in range(4):
                wt = wpool.tile([P, 2, D], FP8, name=f"{name}_wv{kp}")
                nc.sync.dma_start(wt, c.w[f"wv{idx}"][kp, :, :, :])
                wv_tiles.append(wt)
            cvb = None
            if c.use_bias:
                cvb = wpool.tile([P, D], BF16, name=f"{name}_cvb")
                nc.gpsimd.partition_broadcast(cvb, c.bias_sb[f"cv{idx}"])
            psum = sub.enter_context(
                tc.tile_pool(name=f"{name}_vps", bufs=2, space="PSUM"))
            pss = {}
            for t in range(TT):
                for n in range(2):
                    pss[(t, n)] = psum.tile([P, 512], F32, tag=f"vp{n}",
                                            name=f"{name}_vps{t}_{n}")
                for kp in range(4):
                    for n in range(2):
                        nc.tensor.matmul(
                            pss[(t, n)],
                            lhsT=xlnT4[kp][:, :, t * P:(t + 1) * P],
                            rhs=wv_tiles[kp][:, :, n * 512:(n + 1) * 512],
                            start=(kp == 0), stop=(kp == 3), perf_mode=DRm)
                for n in range(2):
                    if c.use_bias:
                        nc.vector.scalar_tensor_tensor(
                            vstage[t][:, n * vhw:(n + 1) * vhw],
                            pss[(t, n)], 0.25,
                            cvb[:, n * 512:(n + 1) * 512],
                            op0=ALU.mult, op1=ALU.add)
                    else:
                        nc.vector.tensor_scalar_mul(
                            vstage[t][:, n * vhw:(n + 1) * vhw],
                            pss[(t, n)], 0.25)
            if c.group > 1:
                v_in = c.v_in[idx - 1]
                for t in range(TT):
                    nc.sync.dma_start(
                        v_in[t * P * vw:(t + 1) * P * vw]
                        .rearrange("(p f) -> p f", f=vw), vstage[t])
                nc.gpsimd.collective_compute(
                    "AllGather", ALU.bypass, replica_groups=RG,
                    ins=[v_in[:]], outs=[c.v_out[idx - 1][:]])

        # --- Q^T projection (overlaps the gathers) ---
        proj_fm(c, ph, f"{name}_qproj", xlnT4, c.w[f"wq{idx}"], qT,
                bias_key=f"cq{idx}", out_scale=1.0 / 256)

        if c.group == 1:
            for m in range(8):
                nc.vector.tensor_copy(kT_m[m], kT_loc[m])
            for t in range(TT):
                for half in (0, 1):
                    dst = vones_hr[half][0][:, t // 2, t % 2, :].rearrange(
                        "p (hh e) -> p hh e", e=P)[:, :, 0:64]
                    nc.scalar.copy(
                        dst,
                        vstage[t][:, half * vhw:(half + 1) * vhw].rearrange(
                            "p (hh e) -> p hh e", e=64))
            stage_stack.close()
        else:
            stage_stack.close()
            def read_k(half):
                k_out = c.k_out[idx - 1]
                for mm in range(4):
                    for r in range(c.group):
                        m = half * 4 + mm
                        nc.sync.dma_start(
                            kT_m[m][:, r * tok:(r + 1) * tok],
                            k_out[(r * 8 + m) * P * tok:
                                  (r * 8 + m + 1) * P * tok]
                            .rearrange("(p f) -> p f", f=tok))

            def read_v(half):
                v_out = c.v_out[idx - 1]
                for r in range(c.group):
                    for t in range(TT):
                        dst = vones_hr[half][r][:, t // 2, t % 2, :].rearrange(
                            "p (hh e) -> p hh e", e=P)[:, :, 0:64]
                        src_ap = v_out[(r * TT + t) * P * vw:
                                       (r * TT + t + 1) * P * vw].rearrange(
                            "(p f) -> p f", f=vw)[
                            :, half * vhw:(half + 1) * vhw].rearrange(
                            "p (hh e) -> p hh e", e=64)
                        nc.sync.dma_start(dst, src_ap)

            read_k(0)
            read_v(0)
            read_k(1)
            read_v(1)

        # prefetch out-projection weights while heads run
        wo_pool = ph.enter_context(tc.tile_pool(name=f"{name}_wop", bufs=1))
        wo_tiles = []
        for kp in range(4):
            wt = wo_pool.tile([P, 2, D], FP8, name=f"{name}_wo{kp}")
            nc.sync.dma_start(wt, c.w[f"wo{idx}"][kp, :, :, :])
            wo_tiles.append(wt)

        # --- attention, head pairs, software-pipelined ---
        attnT_pool = ph.enter_context(tc.tile_pool(name=f"{name}_at", bufs=1))
        attnT4 = [attnT_pool.tile([P, 2, tok], FP8, name=f"{name}_attnT{m}")
                  for m in range(4)]
        NR = JT // 2           # score rounds per head (2 j-tiles per round)

        with ExitStack() as sub:
            psum_sc = sub.enter_context(
                tc.tile_pool(name=f"{name}_psc", bufs=3, space="PSUM"))
            psum_av = sub.enter_context(
                tc.tile_pool(name=f"{name}_pav", bufs=2, space="PSUM"))
            pT_pool = sub.enter_context(
                tc.tile_pool(name=f"{name}_pT", bufs=8))
            pT8_pool = sub.enter_context(
                tc.tile_pool(name=f"{name}_pT8", bufs=8))
            small = sub.enter_context(
                tc.tile_pool(name=f"{name}_small", bufs=4))

            pending_epilogue = None

            def emit_epilogue(ep):
                av_pair, m2 = ep
                for s in range(2):
                    dcp = small.tile([64, tok], F32, tag="dcp",
                                     name=f"{name}_dc{m2}_{s}")
                    nc.vector.tensor_copy(dcp, av_pair[s][64:128, :])
                    rden = small.tile([64, tok], F32, tag="rden",
                                      name=f"{name}_rd{m2}_{s}")
                    nc.vector.reciprocal_approx_fast(rden, dcp)
                    nc.vector.tensor_tensor(
                        attnT4[m2 // 2][s * 64:(s + 1) * 64, m2 % 2, :],
                        av_pair[s][0:64, :], rden, op=ALU.mult)

            def emit_av(av_pair, pp, rr, m):
                r_idx = (2 * rr) // TT if c.group > 1 else 0
                tp = rr % (TT // 2)
                rhs1 = pp[1][:, :, :]
                if rhs1.dtype != FP8:
                    rhs1 = rhs1.bitcast(FP8)
                rhs2 = [pp[0][:, :, :], rhs1]
                for s in range(2):
                    h = 2 * m + s
                    hh = h % 8
                    nc.tensor.matmul(
                        av_pair[s],
                        lhsT=vones_hr[h // 8][r_idx][
                            :, tp, :, hh * P:(hh + 1) * P],
                        rhs=rhs2[s],
                        start=(rr == 0), stop=(rr == NR - 1),
                        perf_mode=DRm)

            for m in range(8):       # head pair (2m, 2m+1)
                av_pair = [psum_av.tile([P, tok], F32, tag="av",
                                        name=f"{name}_av{m}_{s}")
                           for s in range(2)]
                pend = None          # ((p0, p1), r)
                for r in range(NR):
                    ps2 = [psum_sc.tile([P, 2, tok], F32, tag="sc",
                                        name=f"{name}_sc{m}_{r}_{s}")
                           for s in range(2)]
                    for u in range(2):
                        jt = 2 * r + u
                        for s in range(2):
                            po = s * 64
                            nc.tensor.matmul(
                                ps2[s][:, u, :],
                                lhsT=kT_m[m][po:po + 64,
                                             jt * P:(jt + 1) * P],
                                rhs=qT[m][po:po + 64, :],
                                start=True, stop=True)
                    if r == 1 and pending_epilogue is not None:
                        emit_epilogue(pending_epilogue)
                        pending_epilogue = None
                    # softmax exp split across engines: s=0 native on ACT
                    # (fp8 out), s=1 Schraudolph on DVE (int8 = fp8 bits);
                    # every 4th round s=1 also runs on ACT to balance load
                    p0 = pT_pool.tile([P, 2, tok], FP8, tag="pT",
                                      name=f"{name}_p{m}_{r}_0")
                    nc.scalar.activation(p0, ps2[0], AF.Exp, scale=1.0 / 16)
                    if r % 4 == 3:
                        p1 = pT_pool.tile([P, 2, tok], FP8, tag="pT",
                                          name=f"{name}_p{m}_{r}_1")
                        nc.scalar.activation(p1, ps2[1], AF.Exp,
                                             scale=1.0 / 16)
                    else:
                        p1 = pT8_pool.tile([P, 2, tok], I8, tag="pT8",
                                           name=f"{name}_p{m}_{r}_1")
                        nc.vector.tensor_scalar(p1, ps2[1], A_EXP8 / 16,
                                                B_EXP8,
                                                op0=ALU.mult, op1=ALU.add)
                    if pend is not None:
                        emit_av(av_pair, *pend)
                    pend = ((p0, p1), r, m)
                emit_av(av_pair, *pend)
                pending_epilogue = (av_pair, m)
            emit_epilogue(pending_epilogue)

        # --- out projection + residual (weights prefetched pre-heads) ---
        with ExitStack() as sub:
            psum_o = sub.enter_context(
                tc.tile_pool(name=f"{name}_po", bufs=1, space="PSUM"))
            ps_o = {}
            for t in range(TT):
                for n in range(2):
                    ps_o[(t, n)] = psum_o.tile([P, 512], F32, tag=f"o{t}_{n}",
                                               name=f"{name}_pso{t}_{n}")
            for mp in range(4):
                for t in range(TT):
                    for n in range(2):
                        nc.tensor.matmul(
                            ps_o[(t, n)],
                            lhsT=attnT4[mp][:, :, t * P:(t + 1) * P],
                            rhs=wo_tiles[mp][:, :, n * 512:(n + 1) * 512],
                            start=(mp == 0), stop=(mp == 3), perf_mode=DRm)
            bob = None
            if c.use_bias:
                bob = wo_pool.tile([P, D], BF16, name=f"{name}_bob")
                nc.gpsimd.partition_broadcast(bob, c.bias_sb[f"bo{idx}"])
            for t in range(TT):
                for n in range(2):
                    sl = slice(n * 512, (n + 1) * 512)
                    nc.vector.scalar_tensor_tensor(
                        c.xres[t][:, sl], ps_o[(t, n)], 1.0 / 256,
                        c.xres[t][:, sl], op0=ALU.mult, op1=ALU.add)
                if c.use_bias:
                    nc.vector.tensor_add(c.xres[t], c.xres[t], bob)


def emit_ff(c):
    nc, tc = c.nc, c.tc
    name = "ff"
    TT, tok = c.TT, c.tok

    with ExitStack() as ph:
        xlnT = emit_ln_transpose(c, ph, name)

        h2_pool = ph.enter_context(tc.tile_pool(name=f"{name}_h2", bufs=1))
        h2T = [h2_pool.tile([P, tok], BF16, name=f"{name}_h2T{m}")
               for m in range(32)]

        with ExitStack() as sub:
            f1_pool = sub.enter_context(
                tc.tile_pool(name=f"{name}_f1", bufs=6))
            psum_ff = sub.enter_context(
                tc.tile_pool(name=f"{name}_pff", bufs=4, space="PSUM"))
            gl_pool = sub.enter_context(
                tc.tile_pool(name=f"{name}_gl", bufs=3))

            for pm in range(32):
                f1 = f1_pool.tile([P, 8, 256], BF16, tag="f1",
                                  name=f"{name}_f1_{pm}")
                nc.sync.dma_start(
                    f1.rearrange("p a b -> p (a b)"), c.w["wf1"][pm, :, :])
                ps_a = psum_ff.tile([P, tok], F32, tag="ffa",
                                    name=f"{name}_fa{pm}")
                ps_g = psum_ff.tile([P, tok], F32, tag="ffg",
                                    name=f"{name}_fg{pm}")
                for kd in range(8):
                    nc.tensor.matmul(ps_a, lhsT=f1[:, kd, 0:128],
                                     rhs=xlnT[kd], start=(kd == 0),
                                     stop=(kd == 7 and not c.use_bias))
                for kd in range(8):
                    nc.tensor.matmul(ps_g, lhsT=f1[:, kd, 128:256],
                                     rhs=xlnT[kd], start=(kd == 0),
                                     stop=(kd == 7 and not c.use_bias))
                if c.use_bias:
                    nc.tensor.matmul(
                        ps_a, lhsT=c.bias_sb["c1"][0:1, pm * 256:pm * 256 + 128],
                        rhs=c.ones_bf, start=False, stop=True)
                    nc.tensor.matmul(
                        ps_g,
                        lhsT=c.bias_sb["c1"][0:1, pm * 256 + 128:pm * 256 + 256],
                        rhs=c.ones_bf, start=False, stop=True)
                gl = gl_pool.tile([P, tok], BF16, tag="gelu",
                                  name=f"{name}_gl{pm}")
                nc.scalar.activation(gl, ps_g, AF.Gelu)
                nc.vector.tensor_tensor(h2T[pm], ps_a, gl, op=ALU.mult)

        # FF2 + residual
        with ExitStack() as sub:
            wf2_pool = sub.enter_context(
                tc.tile_pool(name=f"{name}_w2", bufs=8))
            psum_o = sub.enter_context(
                tc.tile_pool(name=f"{name}_po2", bufs=1, space="PSUM"))
            ps_o = {}
            for t in range(TT):
                for n in range(2):
                    ps_o[(t, n)] = psum_o.tile([P, 512], F32, tag=f"o{t}_{n}",
                                               name=f"{name}_pso{t}_{n}")
            for m in range(32):
                w2 = wf2_pool.tile([P, D], BF16, tag="w2",
                                   name=f"{name}_w2_{m}")
                nc.sync.dma_start(w2, c.w["wf2"][m * P:(m + 1) * P, :])
                for t in range(TT):
                    for n in range(2):
                        nc.tensor.matmul(
                            ps_o[(t, n)],
                            lhsT=h2T[m][:, t * P:(t + 1) * P],
                            rhs=w2[:, n * 512:(n + 1) * 512],
                            start=(m == 0),
                            stop=(m == 31 and not c.use_bias))
            if c.use_bias:
                for t in range(TT):
                    for n in range(2):
                        nc.tensor.matmul(
                            ps_o[(t, n)], lhsT=c.ones_bf[0:1, 0:P],
                            rhs=c.bias_sb["bf2"][0:1, n * 512:(n + 1) * 512],
                            start=False, stop=True)
            for t in range(TT):
                for n in range(2):
                    sl = slice(n * 512, (n + 1) * 512)
                    nc.vector.tensor_add(c.xres[t][:, sl], c.xres[t][:, sl],
                                         ps_o[(t, n)])


# ---------------- host-side helpers ----------------

def _dr4(w):
    """[D, D] -> [4, P, 2, D]: contraction-dim tile pairs for DoubleRow."""
    return np.ascontiguousarray(w.reshape(4, 2, P, D).transpose(0, 2, 1, 3))


def prep_weights(inp):
    """Fold LN gains + attention scale into DR-packed fp8 weights."""
    f = np.float32
    out = {}
    for i in (1, 2):
        g = np.asarray(inp[f"ln{i}_g"], f)
        out[f"wq{i}"] = _dr4(g[:, None] * np.asarray(inp[f"w_q{i}"], f).T
                             * np.float32(DH ** -0.5 * 64.0))
        out[f"wk{i}"] = _dr4(g[:, None] * np.asarray(inp[f"w_k{i}"], f).T
                             * f(16))
        out[f"wv{i}"] = _dr4(g[:, None] * np.asarray(inp[f"w_v{i}"], f).T
                             * f(16))
        out[f"wo{i}"] = _dr4(np.asarray(inp[f"w_o{i}"], f).T * f(16))
    g3 = np.asarray(inp["ln3_g"], f)
    wf1 = g3[:, None] * np.asarray(inp["w_ff1"], f).T          # [1024, 8192]
    # [kd, p, half, pm, col] -> [pm, p, (kd, half, col)]
    out["wf1"] = (wf1.reshape(8, P, 2, 32, P).transpose(3, 1, 0, 2, 4)
                  .reshape(32, P, 2048))
    out["wf2"] = np.asarray(inp["w_ff2"], f).T                 # [4096, 1024]
    import ml_dtypes
    res = {}
    for k, v in out.items():
        dt = (ml_dtypes.bfloat16 if k in ("wf1", "wf2")
              else ml_dtypes.float8_e4m3fn)
        res[k] = np.ascontiguousarray(v.astype(dt))
    return res


def prep_biases(inp):
    """Bias vectors pushed through the projections (all-zero in practice)."""
    f = np.float32
    out = {}
    fp32_keys = []
    sc = np.float32(DH ** -0.5)
    for i in (1, 2):
        b = np.asarray(inp[f"ln{i}_b"], f)
        out[f"cq{i}"] = (np.asarray(inp[f"w_q{i}"], f) @ b
                         * sc).reshape(8, P).T
        out[f"ck{i}"] = (np.asarray(inp[f"w_k{i}"], f) @ b
                         * 16).reshape(8, P).T
        out[f"cv{i}"] = (np.asarray(inp[f"w_v{i}"], f) @ b * 16)[None, :]
        out[f"bo{i}"] = np.asarray(inp[f"b_o{i}"], f)[None, :]
        fp32_keys += [f"cq{i}", f"ck{i}"]
    b3 = np.asarray(inp["ln3_b"], f)
    c1 = np.asarray(inp["w_ff1"], f) @ b3 + np.asarray(inp["b_ff1"], f)
    # reorder to the paired (a, gate) block layout used by wf1
    out["c1"] = c1.reshape(2, 32, P).transpose(1, 0, 2).reshape(1, 2 * FF)
    out["bf2"] = np.asarray(inp["b_ff2"], f)[None, :]
    import ml_dtypes
    res = {}
    for k, v in out.items():
        dt = np.float32 if k in fp32_keys else ml_dtypes.bfloat16
        res[k] = np.ascontiguousarray(v.astype(dt))
    return res


def any_bias(inp):
    keys = ["ln1_b", "ln2_b", "ln3_b", "b_o1", "b_o2", "b_ff1", "b_ff2"]
    return any(np.any(np.asarray(inp[k]) != 0) for k in keys)


# ======================================================================
# Host-side entry point: kernel(**inputs) -> full output [2, 2048, 1024]
# ======================================================================

_B, _N = 2, 2048
_NCORE = 8
_GROUP = 4
_TOK = _N // _GROUP

_cache = {}


def _get_nc(use_bias):
    key = ("nc", use_bias)
    if key not in _cache:
        _cache[key] = build(group=_GROUP, tok=_TOK, use_bias=use_bias)
    return _cache[key]


def kernel(**inputs):
    from concourse.bass_utils import run_bass_kernel_spmd

    inputs = {k: np.asarray(v) for k, v in inputs.items()}
    use_bias = any_bias(inputs)
    nc = _get_nc(use_bias)
    wdev = prep_weights(inputs)
    if use_bias:
        wdev.update(prep_biases(inputs))

    x = np.asarray(inputs["x"], np.float32)
    in_maps = []
    for core in range(_NCORE):
        b, p = core // _GROUP, core % _GROUP
        xs = np.ascontiguousarray(x[b, p * _TOK:(p + 1) * _TOK, :])
        in_maps.append({"x": xs, **wdev})

    res = run_bass_kernel_spmd(nc, in_maps, list(range(_NCORE)))

    y = np.zeros((_B, _N, D), np.float32)
    for core in range(_NCORE):
        b, p = core // _GROUP, core % _GROUP
        y[b, p * _TOK:(p + 1) * _TOK, :] = res.results[core]["y"]
    return y



# revision 38
# speedup vs baseline: 1.0697x; 1.0697x over previous
"""Trainium2 Bass kernel for nn_BasicTransformerBlock (self-contained).

Sharding: sequence-parallel. 8 cores = 2 batch groups x 4 cores; each core
owns 512 tokens of one batch element. K^T and V are AllGathered (fp8,
x16-scaled) within each 4-core group so every core attends over the full
2048-token context of its batch element.

Key design points per core:
  - LayerNorm gains/scales are folded into the weights host-side; xln is
    produced transposed (PE transpose) in fp8 kd-pair layout [128, 2, 512].
  - All Q/K/V/O projections run as fp8 DoubleRow matmuls (2 contraction
    tiles per instruction); scale factors are folded into the weights and
    undone on the psum->SBUF copies.
  - Scores: per head-pair, s=0/s=1 run as row-tiled concurrent matmul pairs
    (K=64 each, rows 0-63 / 64-127 of the PE array).
  - Softmax exp is split across engines: s=0 on ACT (native Exp -> fp8),
    s=1 on DVE via a Schraudolph bit-trick (int8 output = fp8e4m3 bits of
    exp, tensor_scalar mult+add), with every 4th round's s=1 moved to ACT
    to balance engine load.
  - AV + softmax denominator are fused: V tiles hold per-head
    [V(64) | ones(64)] blocks in DoubleRow j-pair layout, so one fp8 DR
    matmul per (head, j-pair) yields AV in psum rows 0:64 and the
    denominator (x64 copies) in rows 64:128.
  - Epilogue per head: DVE copy (shifted) -> reciprocal_approx_fast ->
    multiply into fp8 attnT pairs; out-projection is fp8 DoubleRow with the
    1/256 net scale folded into the residual add (scalar_tensor_tensor).
  - The GEGLU FF stays bf16 (fp8 there costs ~2e-2 max-rel error, over the
    tolerance); FF1/FF2 are dense bf16 matmul streams.
"""


from contextlib import ExitStack

import numpy as np

import concourse.bass as bass
import concourse.mybir as mybir
import concourse.tile as tile
from concourse.tile_rust import add_dep_helper
from concourse import bacc
from concourse.masks import make_identity

F32 = mybir.dt.float32
BF16 = mybir.dt.bfloat16
I8 = mybir.dt.int8
FP8 = mybir.dt.float8e4
AX = mybir.AxisListType.X
AF = mybir.ActivationFunctionType
ALU = mybir.AluOpType

# Schraudolph fast-exp: int8 bits of fp8e4m3(exp(s)) ~= A_EXP8*s + B_EXP8
A_EXP8 = 8.0 / float(np.log(2.0))
B_EXP8 = 8.0 * (7.0 + 0.215)

D = 1024          # model dim
HEADS = 16
DH = 64
FF = 4096         # ff inner (per half)
EPS = 1e-5
P = 128


class Ctx:
    """Holds the bass handles shared across emit stages."""


def build(group: int, tok: int, use_bias: bool = False):
    """group: cores per batch group (1 = no collective, 4 = real).
    tok: local tokens per core (512)."""
    ntok = group * tok
    TT = tok // P          # local token tiles (4)
    JT = ntok // P         # context token tiles (16 when group=4)

    nc = bacc.Bacc("TRN2", target_bir_lowering=False, debug=False,
                   num_devices=8)

    c = Ctx()
    c.nc = nc
    c.group = group
    c.tok = tok
    c.ntok = ntok
    c.TT = TT
    c.JT = JT
    c.use_bias = use_bias

    # ---- I/O ----
    c.x_in = nc.dram_tensor("x", [tok, D], F32, kind="ExternalInput")
    c.y_out = nc.dram_tensor("y", [tok, D], F32, kind="ExternalOutput")
    w = {}
    for i in (1, 2):
        for nm in ("wq", "wk", "wv", "wo"):
            w[f"{nm}{i}"] = nc.dram_tensor(f"{nm}{i}", [4, P, 2, D], FP8,
                                           kind="ExternalInput")
    w["wf1"] = nc.dram_tensor("wf1", [32, P, 2048], BF16, kind="ExternalInput")
    w["wf2"] = nc.dram_tensor("wf2", [FF, D], BF16, kind="ExternalInput")
    if use_bias:
        # per-projection bias vectors (LN beta pushed through W, plus the
        # projection's own bias where it exists)
        for i in (1, 2):
            for nmv in ("cq", "ck"):
                w[f"{nmv}{i}"] = nc.dram_tensor(f"{nmv}{i}", [P, 8], F32,
                                                kind="ExternalInput")
            for nmv in ("cv", "bo"):
                w[f"{nmv}{i}"] = nc.dram_tensor(f"{nmv}{i}", [1, D], BF16,
                                                kind="ExternalInput")
        w["c1"] = nc.dram_tensor("c1", [1, 2 * FF], BF16, kind="ExternalInput")
        w["bf2"] = nc.dram_tensor("bf2", [1, D], BF16, kind="ExternalInput")
    c.w = w

    with ExitStack() as stack:
        tc = stack.enter_context(tile.TileContext(nc))
        c.tc = tc

        const = stack.enter_context(tc.tile_pool(name="const", bufs=1))
        c.identity = const.tile([P, P], BF16, name="identity")
        make_identity(nc, c.identity)
        c.ones64 = const.tile([P, 64], BF16, name="ones64")
        nc.vector.memset(c.ones64, 1.0)
        c.eps_tile = const.tile([P, 1], F32, name="eps_tile")
        nc.vector.memset(c.eps_tile, EPS)
        if use_bias:
            c.ones_bf = const.tile([1, tok], BF16, name="ones_bf")
            nc.vector.memset(c.ones_bf, 1.0)
            c.bias_sb = {}
            for key, t in w.items():
                if key[:2] in ("cq", "ck"):
                    bt = const.tile([P, 8], F32, name=f"sb_{key}")
                elif key[:2] in ("cv", "bo", "bf"):
                    bt = const.tile([1, D], BF16, name=f"sb_{key}")
                else:
                    continue
                nc.sync.dma_start(bt, t[:, :])
                c.bias_sb[key] = bt
            bt = const.tile([1, 2 * FF], BF16, name="sb_c1")
            nc.sync.dma_start(bt, w["c1"][:, :])
            c.bias_sb["c1"] = bt

        xres_pool = stack.enter_context(tc.tile_pool(name="xres", bufs=1))
        c.xres = [xres_pool.tile([P, D], F32, name=f"xres{t}") for t in range(TT)]
        for t in range(TT):
            nc.sync.dma_start(c.xres[t], c.x_in[t * P:(t + 1) * P, :])

        # DRAM bounce buffers: half-sized K/V allgathers (per attention)
        if group > 1:
            ke = D * tok               # whole K^T
            vh = tok * (HEADS // 2) * DH   # half of V (no ones gathered)
            dram = stack.enter_context(
                tc.tile_pool(name="dram", bufs=1, space="DRAM"))
            c.k_in = [dram.tile([ke], FP8, name=f"k_in{i}")
                      for i in (0, 1)]
            c.k_out = [dram.tile([group * ke], FP8, name=f"k_out{i}")
                       for i in (0, 1)]
            c.v_in = [[dram.tile([vh], FP8, name=f"v_in{i}_{half}")
                       for half in (0, 1)] for i in (0, 1)]
            c.v_out = [[dram.tile([group * vh], FP8, name=f"v_out{i}_{half}")
                        for half in (0, 1)] for i in (0, 1)]

        emit_attn(c, 1)
        emit_attn(c, 2)
        emit_ff(c)

        for t in range(TT):
            nc.sync.dma_start(c.y_out[t * P:(t + 1) * P, :], c.xres[t])

    nc.compile()
    return nc


def emit_ln_transpose(c, outer, name, dtype=BF16, scale=1.0, pairs=False):
    """LayerNorm xres (gain/bias pre-folded into weights) and produce
    xlnT: transposed normalized x, optionally scaled and in kd-pair
    layout ([128, 2, tok] x4) for DoubleRow consumers.
    xlnT tiles live in `outer`; scratch pools are closed on return."""
    nc, tc = c.nc, c.tc
    TT = c.TT

    xlnT_pool = outer.enter_context(tc.tile_pool(name=f"{name}_xlnT", bufs=1))
    if pairs:
        xlnT = [xlnT_pool.tile([P, 2, c.tok], dtype, name=f"{name}_xlnT{d}")
                for d in range(4)]
    else:
        xlnT = [xlnT_pool.tile([P, c.tok], dtype, name=f"{name}_xlnT{d}")
                for d in range(8)]

    with ExitStack() as ph:
        pool = ph.enter_context(tc.tile_pool(name=f"{name}_ln", bufs=2))
        psum_sq = ph.enter_context(
            tc.tile_pool(name=f"{name}_psq", bufs=2, space="PSUM"))
        psum_tr = ph.enter_context(
            tc.tile_pool(name=f"{name}_ptr", bufs=4, space="PSUM"))

        for t in range(TT):
            xr = c.xres[t]
            # var = E[x^2] - mu^2: sum and sumsq run concurrently (DVE / ACT)
            ssum = pool.tile([P, 1], F32, tag="ssum", name=f"{name}_ssum{t}")
            nc.vector.reduce_sum(ssum, xr, axis=AX)
            sq_sink = psum_sq.tile([P, D], F32, tag="sq", name=f"{name}_sq{t}")
            sumsq = pool.tile([P, 1], F32, tag="sumsq", name=f"{name}_vs{t}")
            nc.scalar.activation(sq_sink, xr, AF.Square, accum_out=sumsq)
            mu = pool.tile([P, 1], F32, tag="mu", name=f"{name}_mu{t}")
            nc.vector.tensor_scalar_mul(mu, ssum, 1.0 / D)
            musq = pool.tile([P, 1], F32, tag="musq", name=f"{name}_msq{t}")
            nc.vector.tensor_mul(musq, mu, mu)
            bvar = pool.tile([P, 1], F32, tag="bvar", name=f"{name}_bv{t}")
            nc.vector.tensor_scalar(bvar, musq, -1.0, EPS,
                                    op0=ALU.mult, op1=ALU.add)
            std = pool.tile([P, 1], F32, tag="std", name=f"{name}_std{t}")
            nc.scalar.activation(std, sumsq, AF.Sqrt, bias=bvar,
                                 scale=1.0 / D)
            rstd = pool.tile([P, 1], F32, tag="rstd", name=f"{name}_rstd{t}")
            nc.vector.reciprocal(rstd, std)
            if scale != 1.0:
                nc.vector.tensor_scalar_mul(rstd, rstd, scale)
            nmr = pool.tile([P, 1], F32, tag="nmr", name=f"{name}_nmr{t}")
            nc.vector.tensor_mul(nmr, mu, rstd)
            nc.vector.tensor_scalar_mul(nmr, nmr, -1.0)
            xln = pool.tile([P, D], BF16, tag="xln", name=f"{name}_xln{t}")
            nc.vector.tensor_scalar(xln, xr, rstd, nmr,
                                    op0=ALU.mult, op1=ALU.add)
            for dc in range(8):
                tp = psum_tr.tile([P, P], BF16, tag="tp",
                                  name=f"{name}_tp{t}_{dc}")
                nc.tensor.transpose(tp, xln[:, dc * P:(dc + 1) * P],
                                    c.identity)
                if pairs:
                    dst = xlnT[dc // 2][:, dc % 2, t * P:(t + 1) * P]
                else:
                    dst = xlnT[dc][:, t * P:(t + 1) * P]
                nc.vector.tensor_copy(dst, tp)
    return xlnT


def proj_fm(c, ph, name, xlnT4, w_dram, out_tiles, bias_key=None,
            out_scale=1.0, post_m=None):
    """Feature-major projection via fp8 DoubleRow:
    out^T[m] [128, tok] = out_scale * W-pair.T @ xlnT4 (+ bias)."""
    nc, tc = c.nc, c.tc
    DRm = mybir.MatmulPerfMode.DoubleRow
    with ExitStack() as sub:
        wpool = sub.enter_context(tc.tile_pool(name=f"{name}_w", bufs=1))
        w_tiles = []
        for kp in range(4):
            wt = wpool.tile([P, 2, D], FP8, name=f"{name}_w{kp}")
            nc.sync.dma_start(wt, w_dram[kp, :, :, :])
            w_tiles.append(wt)
        psum = sub.enter_context(
            tc.tile_pool(name=f"{name}_ps", bufs=3, space="PSUM"))
        has_bias = c.use_bias and bias_key is not None
        for m in range(8):
            ps = psum.tile([P, c.tok], F32, tag="proj", name=f"{name}_ps{m}")
            for kp in range(4):
                nc.tensor.matmul(ps,
                                 lhsT=w_tiles[kp][:, :, m * P:(m + 1) * P],
                                 rhs=xlnT4[kp], start=(kp == 0),
                                 stop=(kp == 3), perf_mode=DRm)
            if has_bias:
                nc.vector.tensor_scalar(
                    out_tiles[m], ps, out_scale,
                    c.bias_sb[bias_key][:, m:m + 1],
                    op0=ALU.mult, op1=ALU.add)
            else:
                nc.vector.tensor_scalar_mul(out_tiles[m], ps, out_scale)
            if post_m is not None:
                post_m(m)


def emit_attn(c, idx):
    nc, tc = c.nc, c.tc
    name = f"a{idx}"
    TT, JT, tok = c.TT, c.JT, c.tok
    RG = [[0, 1, 2, 3], [4, 5, 6, 7]]
    vw = HEADS * DH          # vstage: 16 plain head blocks
    vhw = (HEADS // 2) * DH  # gathered width per head-half (no ones)

    with ExitStack() as ph:
        xlnT4 = emit_ln_transpose(c, ph, name, dtype=FP8, scale=4.0,
                                  pairs=True)

        kfull_pool = ph.enter_context(tc.tile_pool(name=f"{name}_kf", bufs=1))
        kT_m = [kfull_pool.tile([P, c.ntok], FP8, name=f"{name}_kTm{m}")
                for m in range(8)]
        # V tiles: per head a [V(64) | ones(64)] 128-col block, j-tile
        # pairs interleaved on the ko dim for DoubleRow AV: one DR matmul
        # per (head, j-pair) computes AV (out rows 0:64) AND the softmax
        # denominator x64 (out rows 64:128).
        vones_hr = [[kfull_pool.tile([P, TT // 2, 2, 8 * P], FP8,
                                     name=f"{name}_vo{h}_{r}")
                     for r in range(c.group if c.group > 1 else 1)]
                    for h in (0, 1)]
        qT = [kfull_pool.tile([P, tok], BF16, name=f"{name}_qT{m}")
              for m in range(8)]
        for h in (0, 1):
            for r in range(c.group if c.group > 1 else 1):
                for tp in range(TT // 2):
                    for ko in range(2):
                        nc.gpsimd.memset(
                            vones_hr[h][r][:, tp, ko, :].rearrange(
                                "p (hh e) -> p hh e", e=P)[:, :, 64:128],
                            1.0)

        stage_stack = ExitStack()
        kv_pool = stage_stack.enter_context(
            tc.tile_pool(name=f"{name}_kv", bufs=1))
        kT_loc = [kv_pool.tile([P, tok], FP8, name=f"{name}_kTl{m}")
                  for m in range(8)]
        vstage = [kv_pool.tile([P, vw], FP8, name=f"{name}_vst{t}")
                  for t in range(TT)]

        # --- K^T projection; kick the K allgather when it completes ---
        proj_fm(c, ph, f"{name}_kproj", xlnT4, c.w[f"wk{idx}"], kT_loc,
                bias_key=f"ck{idx}", out_scale=0.25)
        if c.group > 1:
            k_in = c.k_in[idx - 1]
            for m in range(8):
                nc.sync.dma_start(
                    k_in[m * P * tok:(m + 1) * P * tok]
                    .rearrange("(p f) -> p f", f=tok), kT_loc[m])
            nc.gpsimd.collective_compute(
                "AllGather", ALU.bypass, replica_groups=RG,
                ins=[c.k_in[idx - 1][:]], outs=[c.k_out[idx - 1][:]])

        # --- V(+ones) projection, n-major so head-half 0 finishes first ---
        DRm = mybir.MatmulPerfMode.DoubleRow
        with ExitStack() as sub:
            wpool = sub.enter_context(tc.tile_pool(name=f"{name}_wvp", bufs=1))
            wv_tiles = []
            for kp in range(4):
                wt = wpool.tile([P, 2, D], FP8, name=f"{name}_wv{kp}")
                nc.sync.dma_start(wt, c.w[f"wv{idx}"][kp, :, :, :])
                wv_tiles.append(wt)
            cvb = None
            if c.use_bias:
                cvb = wpool.tile([P, D], BF16, name=f"{name}_cvb")
                nc.gpsimd.partition_broadcast(cvb, c.bias_sb[f"cv{idx}"])
            psum = sub.enter_context(
                tc.tile_pool(name=f"{name}_vps", bufs=2, space="PSUM"))
            pss = {}
            for t in range(TT):
                for n in range(2):
                    pss[(t, n)] = psum.tile([P, 512], F32, tag=f"vp{n}",
                                            name=f"{name}_vps{t}_{n}")
                for kp in range(4):
                    for n in range(2):
                        nc.tensor.matmul(
                            pss[(t, n)],
                            lhsT=xlnT4[kp][:, :, t * P:(t + 1) * P],
                            rhs=wv_tiles[kp][:, :, n * 512:(n + 1) * 512],
                            start=(kp == 0), stop=(kp == 3), perf_mode=DRm)
                for n in range(2):
                    if c.use_bias:
                        nc.vector.scalar_tensor_tensor(
                            vstage[t][:, n * vhw:(n + 1) * vhw],
                            pss[(t, n)], 0.25,
                            cvb[:, n * 512:(n + 1) * 512],
                            op0=ALU.mult, op1=ALU.add)
                    else:
                        nc.vector.tensor_scalar_mul(
                            vstage[t][:, n * vhw:(n + 1) * vhw],
                            pss[(t, n)], 0.25)
            if c.group > 1:
                for n in range(2):
                    v_in = c.v_in[idx - 1][n]
                    for t in range(TT):
                        nc.sync.dma_start(
                            v_in[t * P * vhw:(t + 1) * P * vhw]
                            .rearrange("(p f) -> p f", f=vhw),
                            vstage[t][:, n * vhw:(n + 1) * vhw])
                    if n == 0:
                        nc.gpsimd.collective_compute(
                            "AllGather", ALU.bypass, replica_groups=RG,
                            ins=[v_in[:]], outs=[c.v_out[idx - 1][0][:]])

        # --- Q^T projection (overlaps the gathers) ---
        proj_fm(c, ph, f"{name}_qproj", xlnT4, c.w[f"wq{idx}"], qT,
                bias_key=f"cq{idx}", out_scale=1.0 / 256)

        if c.group == 1:
            for m in range(8):
                nc.vector.tensor_copy(kT_m[m], kT_loc[m])
            for t in range(TT):
                for half in (0, 1):
                    dst = vones_hr[half][0][:, t // 2, t % 2, :].rearrange(
                        "p (hh e) -> p hh e", e=P)[:, :, 0:64]
                    nc.scalar.copy(
                        dst,
                        vstage[t][:, half * vhw:(half + 1) * vhw].rearrange(
                            "p (hh e) -> p hh e", e=64))
            stage_stack.close()
        else:
            # remaining V half-gather, then read everything back.
            nc.gpsimd.collective_compute(
                "AllGather", ALU.bypass, replica_groups=RG,
                ins=[c.v_in[idx - 1][1][:]], outs=[c.v_out[idx - 1][1][:]])
            stage_stack.close()
            def read_k(half):
                k_out = c.k_out[idx - 1]
                for mm in range(4):
                    for r in range(c.group):
                        m = half * 4 + mm
                        nc.sync.dma_start(
                            kT_m[m][:, r * tok:(r + 1) * tok],
                            k_out[(r * 8 + m) * P * tok:
                                  (r * 8 + m + 1) * P * tok]
                            .rearrange("(p f) -> p f", f=tok))

            def read_v(half):
                v_out = c.v_out[idx - 1][half]
                for r in range(c.group):
                    for t in range(TT):
                        dst = vones_hr[half][r][:, t // 2, t % 2, :].rearrange(
                            "p (hh e) -> p hh e", e=P)[:, :, 0:64]
                        nc.sync.dma_start(
                            dst,
                            v_out[(r * TT + t) * P * vhw:
                                  (r * TT + t + 1) * P * vhw]
                            .rearrange("(p f) -> p (f)", f=vhw)
                            .rearrange("p (hh e) -> p hh e", e=64))

            read_k(0)
            read_v(0)
            read_k(1)
            read_v(1)

        # prefetch out-projection weights while heads run
        wo_pool = ph.enter_context(tc.tile_pool(name=f"{name}_wop", bufs=1))
        wo_tiles = []
        for kp in range(4):
            wt = wo_pool.tile([P, 2, D], FP8, name=f"{name}_wo{kp}")
            nc.sync.dma_start(wt, c.w[f"wo{idx}"][kp, :, :, :])
            wo_tiles.append(wt)

        # --- attention, head pairs, software-pipelined ---
        attnT_pool = ph.enter_context(tc.tile_pool(name=f"{name}_at", bufs=1))
        attnT4 = [attnT_pool.tile([P, 2, tok], FP8, name=f"{name}_attnT{m}")
                  for m in range(4)]
        NR = JT // 2           # score rounds per head (2 j-tiles per round)

        with ExitStack() as sub:
            psum_sc = sub.enter_context(
                tc.tile_pool(name=f"{name}_psc", bufs=3, space="PSUM"))
            psum_av = sub.enter_context(
                tc.tile_pool(name=f"{name}_pav", bufs=2, space="PSUM"))
            pT_pool = sub.enter_context(
                tc.tile_pool(name=f"{name}_pT", bufs=8))
            pT8_pool = sub.enter_context(
                tc.tile_pool(name=f"{name}_pT8", bufs=8))
            small = sub.enter_context(
                tc.tile_pool(name=f"{name}_small", bufs=4))

            pending_epilogue = None

            def emit_epilogue(ep):
                av_pair, m2 = ep
                for s in range(2):
                    dcp = small.tile([64, tok], F32, tag="dcp",
                                     name=f"{name}_dc{m2}_{s}")
                    nc.vector.tensor_copy(dcp, av_pair[s][64:128, :])
                    rden = small.tile([64, tok], F32, tag="rden",
                                      name=f"{name}_rd{m2}_{s}")
                    nc.vector.reciprocal_approx_fast(rden, dcp)
                    nc.vector.tensor_tensor(
                        attnT4[m2 // 2][s * 64:(s + 1) * 64, m2 % 2, :],
                        av_pair[s][0:64, :], rden, op=ALU.mult)

            def emit_av(av_pair, pp, rr, m):
                r_idx = (2 * rr) // TT if c.group > 1 else 0
                tp = rr % (TT // 2)
                rhs1 = pp[1][:, :, :]
                if rhs1.dtype != FP8:
                    rhs1 = rhs1.bitcast(FP8)
                rhs2 = [pp[0][:, :, :], rhs1]
                for s in range(2):
                    h = 2 * m + s
                    hh = h % 8
                    nc.tensor.matmul(
                        av_pair[s],
                        lhsT=vones_hr[h // 8][r_idx][
                            :, tp, :, hh * P:(hh + 1) * P],
                        rhs=rhs2[s],
                        start=(rr == 0), stop=(rr == NR - 1),
                        perf_mode=DRm)

            for m in range(8):       # head pair (2m, 2m+1)
                av_pair = [psum_av.tile([P, tok], F32, tag="av",
                                        name=f"{name}_av{m}_{s}")
                           for s in range(2)]
                pend = None          # ((p0, p1), r)
                for r in range(NR):
                    ps2 = [psum_sc.tile([P, 2, tok], F32, tag="sc",
                                        name=f"{name}_sc{m}_{r}_{s}")
                           for s in range(2)]
                    for u in range(2):
                        jt = 2 * r + u
                        for s in range(2):
                            po = s * 64
                            nc.tensor.matmul(
                                ps2[s][:, u, :],
                                lhsT=kT_m[m][po:po + 64,
                                             jt * P:(jt + 1) * P],
                                rhs=qT[m][po:po + 64, :],
                                start=True, stop=True)
                    if r == 1 and pending_epilogue is not None:
                        emit_epilogue(pending_epilogue)
                        pending_epilogue = None
                    # softmax exp split across engines: s=0 native on ACT
                    # (fp8 out), s=1 Schraudolph on DVE (int8 = fp8 bits);
                    # every 4th round s=1 also runs on ACT to balance load
                    p0 = pT_pool.tile([P, 2, tok], FP8, tag="pT",
                                      name=f"{name}_p{m}_{r}_0")
                    nc.scalar.activation(p0, ps2[0], AF.Exp, scale=1.0 / 16)
                    if r % 4 == 3:
                        p1 = pT_pool.tile([P, 2, tok], FP8, tag="pT",
                                          name=f"{name}_p{m}_{r}_1")
                        nc.scalar.activation(p1, ps2[1], AF.Exp,
                                             scale=1.0 / 16)
                    else:
                        p1 = pT8_pool.tile([P, 2, tok], I8, tag="pT8",
                                           name=f"{name}_p{m}_{r}_1")
                        nc.vector.tensor_scalar(p1, ps2[1], A_EXP8 / 16,
                                                B_EXP8,
                                                op0=ALU.mult, op1=ALU.add)
                    if pend is not None:
                        emit_av(av_pair, *pend)
                    pend = ((p0, p1), r, m)
                emit_av(av_pair, *pend)
                pending_epilogue = (av_pair, m)
            emit_epilogue(pending_epilogue)

        # --- out projection + residual (weights prefetched pre-heads) ---
        with ExitStack() as sub:
            psum_o = sub.enter_context(
                tc.tile_pool(name=f"{name}_po", bufs=1, space="PSUM"))
            ps_o = {}
            for t in range(TT):
                for n in range(2):
                    ps_o[(t, n)] = psum_o.tile([P, 512], F32, tag=f"o{t}_{n}",
                                               name=f"{name}_pso{t}_{n}")
            for mp in range(4):
                for t in range(TT):
                    for n in range(2):
                        nc.tensor.matmul(
                            ps_o[(t, n)],
                            lhsT=attnT4[mp][:, :, t * P:(t + 1) * P],
                            rhs=wo_tiles[mp][:, :, n * 512:(n + 1) * 512],
                            start=(mp == 0), stop=(mp == 3), perf_mode=DRm)
            bob = None
            if c.use_bias:
                bob = wo_pool.tile([P, D], BF16, name=f"{name}_bob")
                nc.gpsimd.partition_broadcast(bob, c.bias_sb[f"bo{idx}"])
            for t in range(TT):
                for n in range(2):
                    sl = slice(n * 512, (n + 1) * 512)
                    nc.vector.scalar_tensor_tensor(
                        c.xres[t][:, sl], ps_o[(t, n)], 1.0 / 256,
                        c.xres[t][:, sl], op0=ALU.mult, op1=ALU.add)
                if c.use_bias:
                    nc.vector.tensor_add(c.xres[t], c.xres[t], bob)


def emit_ff(c):
    nc, tc = c.nc, c.tc
    name = "ff"
    TT, tok = c.TT, c.tok

    with ExitStack() as ph:
        xlnT = emit_ln_transpose(c, ph, name)

        h2_pool = ph.enter_context(tc.tile_pool(name=f"{name}_h2", bufs=1))
        h2T = [h2_pool.tile([P, tok], BF16, name=f"{name}_h2T{m}")
               for m in range(32)]

        with ExitStack() as sub:
            f1_pool = sub.enter_context(
                tc.tile_pool(name=f"{name}_f1", bufs=6))
            psum_ff = sub.enter_context(
                tc.tile_pool(name=f"{name}_pff", bufs=3, space="PSUM"))
            gl_pool = sub.enter_context(
                tc.tile_pool(name=f"{name}_gl", bufs=3))

            for pm in range(32):
                f1 = f1_pool.tile([P, 8, 256], BF16, tag="f1",
                                  name=f"{name}_f1_{pm}")
                nc.sync.dma_start(
                    f1.rearrange("p a b -> p (a b)"), c.w["wf1"][pm, :, :])
                ps_a = psum_ff.tile([P, tok], F32, tag="ffa",
                                    name=f"{name}_fa{pm}")
                ps_g = psum_ff.tile([P, tok], F32, tag="ffg",
                                    name=f"{name}_fg{pm}")
                for kd in range(8):
                    nc.tensor.matmul(ps_a, lhsT=f1[:, kd, 0:128],
                                     rhs=xlnT[kd], start=(kd == 0),
                                     stop=(kd == 7 and not c.use_bias))
                for kd in range(8):
                    nc.tensor.matmul(ps_g, lhsT=f1[:, kd, 128:256],
                                     rhs=xlnT[kd], start=(kd == 0),
                                     stop=(kd == 7 and not c.use_bias))
                if c.use_bias:
                    nc.tensor.matmul(
                        ps_a, lhsT=c.bias_sb["c1"][0:1, pm * 256:pm * 256 + 128],
                        rhs=c.ones_bf, start=False, stop=True)
                    nc.tensor.matmul(
                        ps_g,
                        lhsT=c.bias_sb["c1"][0:1, pm * 256 + 128:pm * 256 + 256],
                        rhs=c.ones_bf, start=False, stop=True)
                gl = gl_pool.tile([P, tok], BF16, tag="gelu",
                                  name=f"{name}_gl{pm}")
                nc.scalar.activation(gl, ps_g, AF.Gelu)
                nc.vector.tensor_tensor(h2T[pm], ps_a, gl, op=ALU.mult)

        # FF2 + residual
        with ExitStack() as sub:
            wf2_pool = sub.enter_context(
                tc.tile_pool(name=f"{name}_w2", bufs=8))
            psum_o = sub.enter_context(
                tc.tile_pool(name=f"{name}_po2", bufs=1, space="PSUM"))
            ps_o = {}
            for t in range(TT):
                for n in range(2):
                    ps_o[(t, n)] = psum_o.tile([P, 512], F32, tag=f"o{t}_{n}",
                                               name=f"{name}_pso{t}_{n}")
            for m in range(32):
                w2 = wf2_pool.tile([P, D], BF16, tag="w2",
                                   name=f"{name}_w2_{m}")
                nc.sync.dma_start(w2, c.w["wf2"][m * P:(m + 1) * P, :])
                for t in range(TT):
                    for n in range(2):
                        nc.tensor.matmul(
                            ps_o[(t, n)],
                            lhsT=h2T[m][:, t * P:(t + 1) * P],
                            rhs=w2[:, n * 512:(n + 1) * 512],
                            start=(m == 0),
                            stop=(m == 31 and not c.use_bias))
            if c.use_bias:
                for t in range(TT):
                    for n in range(2):
                        nc.tensor.matmul(
                            ps_o[(t, n)], lhsT=c.ones_bf[0:1, 0:P],
                            rhs=c.bias_sb["bf2"][0:1, n * 512:(n + 1) * 512],
                            start=False, stop=True)
            for t in range(TT):
                for n in range(2):
                    sl = slice(n * 512, (n + 1) * 512)
                    nc.vector.tensor_add(c.xres[t][:, sl], c.xres[t][:, sl],
                                         ps_o[(t, n)])


# ---------------- host-side helpers ----------------

def _dr4(w):
    """[D, D] -> [4, P, 2, D]: contraction-dim tile pairs for DoubleRow."""
    return np.ascontiguousarray(w.reshape(4, 2, P, D).transpose(0, 2, 1, 3))


def prep_weights(inp):
    """Fold LN gains + attention scale into DR-packed fp8 weights."""
    f = np.float32
    out = {}
    for i in (1, 2):
        g = np.asarray(inp[f"ln{i}_g"], f)
        out[f"wq{i}"] = _dr4(g[:, None] * np.asarray(inp[f"w_q{i}"], f).T
                             * np.float32(DH ** -0.5 * 64.0))
        out[f"wk{i}"] = _dr4(g[:, None] * np.asarray(inp[f"w_k{i}"], f).T
                             * f(16))
        out[f"wv{i}"] = _dr4(g[:, None] * np.asarray(inp[f"w_v{i}"], f).T
                             * f(16))
        out[f"wo{i}"] = _dr4(np.asarray(inp[f"w_o{i}"], f).T * f(16))
    g3 = np.asarray(inp["ln3_g"], f)
    wf1 = g3[:, None] * np.asarray(inp["w_ff1"], f).T          # [1024, 8192]
    # [kd, p, half, pm, col] -> [pm, p, (kd, half, col)]
    out["wf1"] = (wf1.reshape(8, P, 2, 32, P).transpose(3, 1, 0, 2, 4)
                  .reshape(32, P, 2048))
    out["wf2"] = np.asarray(inp["w_ff2"], f).T                 # [4096, 1024]
    import ml_dtypes
    res = {}
    for k, v in out.items():
        dt = (ml_dtypes.bfloat16 if k in ("wf1", "wf2")
              else ml_dtypes.float8_e4m3fn)
        res[k] = np.ascontiguousarray(v.astype(dt))
    return res


def prep_biases(inp):
    """Bias vectors pushed through the projections (all-zero in practice)."""
    f = np.float32
    out = {}
    fp32_keys = []
    sc = np.float32(DH ** -0.5)
    for i in (1, 2):
        b = np.asarray(inp[f"ln{i}_b"], f)
        out[f"cq{i}"] = (np.asarray(inp[f"w_q{i}"], f) @ b
                         * sc).reshape(8, P).T
        out[f"ck{i}"] = (np.asarray(inp[f"w_k{i}"], f) @ b
                         * 16).reshape(8, P).T
        out[f"cv{i}"] = (np.asarray(inp[f"w_v{i}"], f) @ b * 16)[None, :]
        out[f"bo{i}"] = np.asarray(inp[f"b_o{i}"], f)[None, :]
        fp32_keys += [f"cq{i}", f"ck{i}"]
    b3 = np.asarray(inp["ln3_b"], f)
    c1 = np.asarray(inp["w_ff1"], f) @ b3 + np.asarray(inp["b_ff1"], f)
    # reorder to the paired (a, gate) block layout used by wf1
    out["c1"] = c1.reshape(2, 32, P).transpose(1, 0, 2).reshape(1, 2 * FF)
    out["bf2"] = np.asarray(inp["b_ff2"], f)[None, :]
    import ml_dtypes
    res = {}
    for k, v in out.items():
        dt = np.float32 if k in fp32_keys else ml_dtypes.bfloat16
        res[k] = np.ascontiguousarray(v.astype(dt))
    return res


def any_bias(inp):
    keys = ["ln1_b", "ln2_b", "ln3_b", "b_o1", "b_o2", "b_ff1", "b_ff2"]
    return any(np.any(np.asarray(inp[k]) != 0) for k in keys)


# ======================================================================
# Host-side entry point: kernel(**inputs) -> full output [2, 2048, 1024]
# ======================================================================

_B, _N = 2, 2048
_NCORE = 8
_GROUP = 4
_TOK = _N // _GROUP

_cache = {}


def _get_nc(use_bias):
    key = ("nc", use_bias)
    if key not in _cache:
        _cache[key] = build(group=_GROUP, tok=_TOK, use_bias=use_bias)
    return _cache[key]


def kernel(**inputs):
    from concourse.bass_utils import run_bass_kernel_spmd

    inputs = {k: np.asarray(v) for k, v in inputs.items()}
    use_bias = any_bias(inputs)
    nc = _get_nc(use_bias)
    wdev = prep_weights(inputs)
    if use_bias:
        wdev.update(prep_biases(inputs))

    x = np.asarray(inputs["x"], np.float32)
    in_maps = []
    for core in range(_NCORE):
        b, p = core // _GROUP, core % _GROUP
        xs = np.ascontiguousarray(x[b, p * _TOK:(p + 1) * _TOK, :])
        in_maps.append({"x": xs, **wdev})

    res = run_bass_kernel_spmd(nc, in_maps, list(range(_NCORE)))

    y = np.zeros((_B, _N, D), np.float32)
    for core in range(_NCORE):
        b, p = core // _GROUP, core % _GROUP
        y[b, p * _TOK:(p + 1) * _TOK, :] = res.results[core]["y"]
    return y

